# revision 46
# baseline (speedup 1.0000x reference)
"""Trainium2 Bass kernel for causal self-attention (dense transformer block attn).

Reference computation (per batch b):
    qkv = x @ W_attn + b_attn ; split into per-head Q, K, V (16 heads, hs=64)
    att = softmax(mask(Q K^T / sqrt(hs))) ; y = att @ V ; out = y @ W_proj + b_proj

Sharding (8 cores): data parallel on B (2) x tensor parallel on head groups
(4 groups of 4 heads, Megatron-style column/row split of W_attn / W_proj).
Each core computes a partial out^T [1024, 2048] (bf16); host sums the 4
partials per batch, adds b_proj and transposes.

Core kernel layout notes:
  - Everything on-chip is transposed: x^T, qkv^T ([feature, T]), scores are
    computed as S^T = K Q^T with k-positions on partitions so that the PV
    matmul needs no transposes (P^T is the moving operand, V natural the
    stationary).
  - Softmax denominator: the PV stationary is [V | ones] (or [ones | V]) so
    the other 64 partitions of the PV psum accumulate 64 copies of
    sum_k P[q,k]. A single custom-DVE reciprocal_approx_fast reads those
    partitions and writes the reciprocal onto the y-row partition half
    (64-channel DVE ops may write either partition half regardless of the
    source half), then one tensor_mul normalizes during psum evacuation.
  - exp() runs on ScalarE straight out of PSUM in wide [128, 2, <=512]
    instructions (two heads at once) to amortize the ACT overhead.
  - The PE p-state ramp (0.65 -> 1.2 -> 2.4 GHz after 3us of *continuous*
    work) makes every PE idle gap cost ~2x its length: the schedule keeps
    the PE queue dense front-to-back and warms the array with junk matmuls
    while the first input DMAs land.
"""

import numpy as np
import ml_dtypes

import concourse.bass as bass
import concourse.tile as tile
import concourse.mybir as mybir
from concourse import bacc
from concourse.bass_utils import run_bass_kernel_spmd

BF16 = mybir.dt.bfloat16
F32 = mybir.dt.float32
AF = mybir.ActivationFunctionType

T = 2048          # sequence length
C = 1024          # model dim
HPC = 4           # heads per core
HS = 64           # head size
NF = 3 * HPC * HS  # per-core qkv features (768)
N_CORES = 8
QB = 512          # q block (psum bank of f32)

bf16 = ml_dtypes.bfloat16

_DEBUG_DUMPS = False


def build_kernel():
    nc = bacc.Bacc("TRN2", target_bir_lowering=False, debug=False)

    xT = nc.dram_tensor("xT", [C, T], BF16, kind="ExternalInput").ap()
    W = nc.dram_tensor("W", [C, NF], BF16, kind="ExternalInput").ap()
    bcols = nc.dram_tensor("bcols", [128, 6], F32, kind="ExternalInput").ap()
    Wp = nc.dram_tensor("Wp", [HPC * HS, C], BF16, kind="ExternalInput").ap()
    mask = nc.dram_tensor("mask", [128, 128], BF16, kind="ExternalInput").ap()
    ident = nc.dram_tensor("ident", [128, 128], BF16, kind="ExternalInput").ap()
    outT = nc.dram_tensor("outT", [C, T], BF16, kind="ExternalOutput").ap()

    with tile.TileContext(nc) as tc:
        _emit(nc, tc, xT, W, bcols, Wp, mask, ident, outT)
    nc.compile()
    return nc


def _emit(nc, tc, xT, W, bcols, Wp, mask, ident, outT):
    from contextlib import ExitStack

    ctx = ExitStack()
    consts = ctx.enter_context(tc.tile_pool(name="consts", bufs=1))
    pt_pool = ctx.enter_context(tc.tile_pool(name="pt", bufs=1))
    rt_pool = ctx.enter_context(tc.tile_pool(name="rt", bufs=2))
    # 4 bufs: an ob tile is held until its output DMA transfer completes
    # (~1-2.5us), so depth 2 would pace the projection evacuations
    osb_pool = ctx.enter_context(tc.tile_pool(name="osb", bufs=4))
    ps_s = ctx.enter_context(tc.tile_pool(name="ps_s", bufs=2, space="PSUM"))
    ps_pv = ctx.enter_context(tc.tile_pool(name="ps_pv", bufs=2, space="PSUM"))
    ps_q = ctx.enter_context(tc.tile_pool(name="ps_q", bufs=2, space="PSUM"))

    # ---------------- constant / input loads ----------------
    xT_v = xT.rearrange("(c p) t -> p c t", p=128)
    xT_t = consts.tile([128, 8, T], BF16, tag="xT", name="xT_t")
    W_v = W.rearrange("(c p) n -> p c n", p=128)
    W_t = consts.tile([128, 8, NF], BF16, tag="W", name="W_t")
    mask_t = consts.tile([128, 128], BF16, tag="mask", name="mask_t")
    ident_t = consts.tile([128, 128], BF16, tag="ident", name="id_t")
    b_t = consts.tile([128, 6], F32, tag="b", name="b_t")
    Wp_t = consts.tile([128, 2, C], BF16, tag="Wp", name="Wp_t")

    qkvT = consts.tile([128, 6, T], BF16, tag="qkvT", name="qkvT")
    # vnat[p, pair, j, hl, col]: PV stationary tiles. hl=0: [V | ones],
    # hl=1: [ones | V] so that y lands on the partitions matching yT layout.
    vnat = consts.tile([128, 2, 16, 2, 128], BF16, tag="vnat", name="vnat")
    yT = consts.tile([128, 2, T], BF16, tag="yT", name="yT")

    # warm up the ACT exp table early so the ~1.3us load overlaps the lead-in
    warm = consts.tile([128, 8], F32, tag="warm", name="warm")
    nc.vector.memset(warm, 0.0)
    nc.scalar.activation(warm, warm, AF.Exp, scale=1.0)
    wtile = consts.tile([128, QB], BF16, tag="wtile", name="wtile")
    nc.vector.memset(wtile, 0.0)
    nc.vector.memset(vnat[:, :, :, 0, 64:128], 1.0)
    nc.vector.memset(vnat[:, :, :, 1, 0:64], 1.0)

    # The Scalar queue's DMA path is much slower than Sync/GpSimd and its
    # ring blocks later issues, so it only carries the tiny constants.
    nc.scalar.dma_start(out=mask_t, in_=mask)
    nc.scalar.dma_start(out=ident_t, in_=ident)
    nc.scalar.dma_start(out=b_t, in_=bcols)
    # Input streaming, ordered by first use. W is pair-major (host-side
    # column regroup): its first half (Q0/K0/V0) plus x[qb0] gate wave 0;
    # everything else overlaps compute. Sync takes c chunks 0-3, GpSimd
    # 4-7, both walking [W-half0, x qb0, x qb1, W-half1, x qb2, x qb3].
    CG = [(0, 4), (4, 8)]
    QUEUES = [nc.sync, nc.gpsimd]

    def xdma(c0, c1, qb4, q):
        q.dma_start(out=xT_t[:, c0:c1, qb4 * QB:(qb4 + 1) * QB],
                    in_=xT_v[:, c0:c1, qb4 * QB:(qb4 + 1) * QB])

    for (c0, c1), q in zip(CG, QUEUES):
        q.dma_start(out=W_t[:, c0:c1, 0:NF // 2], in_=W_v[:, c0:c1, 0:NF // 2])
        xdma(c0, c1, 0, q)
        xdma(c0, c1, 1, q)
        q.dma_start(out=W_t[:, c0:c1, NF // 2:NF], in_=W_v[:, c0:c1, NF // 2:NF])
        xdma(c0, c1, 2, q)
        xdma(c0, c1, 3, q)
    # Wp is only needed at the first proj (~halfway in); the slow Scalar
    # DMA path delivers it comfortably by then without loading Sync/GpSimd
    nc.scalar.dma_start(out=Wp_t, in_=Wp.rearrange("(k p) n -> p k n", p=128))

    # ~7us of narrow junk matmuls: the PE p-state ramps to full clock while
    # the first input wave (W + x qb0) lands, and the fine granularity
    # (107ns each at full clock) means the first real matmul preempts with
    # negligible delay.
    junk = ps_q.tile([128, QB], F32, tag="q", name="junk")
    for _ in range(51):
        nc.tensor.matmul(junk[:, 0:256], lhsT=wtile[:, 0:128],
                         rhs=wtile[:, 0:256], start=True, stop=True)

    # ---------------- phase helpers ----------------
    def qkv_part(nf, qb4):
        # one q block of qkv^T[nf*128:(nf+1)*128, :]  (+ bias on evac)
        ps = ps_q.tile([128, QB], F32, tag="q", name="ps_qkv")
        for c in range(8):
            nc.tensor.matmul(
                ps,
                lhsT=W_t[:, c, nf * 128:(nf + 1) * 128],
                rhs=xT_t[:, c, qb4 * QB:(qb4 + 1) * QB],
                start=(c == 0),
                stop=(c == 7),
            )
        nc.vector.tensor_scalar_add(
            qkvT[:, nf, qb4 * QB:(qb4 + 1) * QB], ps, b_t[:, nf:nf + 1]
        )

    def vtrans1(p, jt):
        # one V^T chunk (qkvT[:, 3p+2, jt]) -> natural V in vnat[:, p, jt]
        pst = ps_q.tile([128, 128], BF16, tag="q", name="ps_vt")
        nc.tensor.transpose(pst, qkvT[:, 3 * p + 2, jt * 128:(jt + 1) * 128], ident_t)
        # single strided copy: psum cols [0:64|64:128] -> vnat
        # [jt, 0, 0:64] and [jt, 1, 64:128]
        v0 = vnat[:, p, jt, 0, 0:64]
        dst = bass.AP(tensor=v0.tensor, offset=v0.offset,
                      ap=[v0.ap[0], [192, 2], [1, 64]])
        s0 = pst[:, 0:64]
        src = bass.AP(tensor=s0.tensor, offset=s0.offset,
                      ap=[s0.ap[0], [64, 2], [1, 64]])
        nc.vector.tensor_copy(dst, src)

    pt_tiles = {}

    def score_block(p, j, qh):
        # one [128 k, <=512 q] score block of S^T chunk j (both heads),
        # exp'd into the (p, j) PT tile. Blocks of a chunk may be emitted
        # across different q waves (pair0) or together (pair1).
        wj = T - 128 * j
        if (p, j) not in pt_tiles:
            pt_tiles[(p, j)] = pt_pool.tile([128, 2, wj], BF16, tag=f"pt{j}",
                                            name=f"pt_{p}_{j}", bufs=1)
        pt = pt_tiles[(p, j)]
        qlo = max(128 * j, 512 * qh)
        qhi = 512 * (qh + 1)
        if qlo >= qhi:
            return
        lo = qlo - 512 * qh
        ps = ps_s.tile([128, 2, 512], F32, tag="s", name="ps_s_t")
        for hl in range(2):
            nc.tensor.matmul(
                ps[:, hl, lo:(qhi - 512 * qh)],
                lhsT=qkvT[64 * hl:64 * hl + 64, 3 * p + 1, j * 128:(j + 1) * 128],
                rhs=qkvT[64 * hl:64 * hl + 64, 3 * p, qlo:qhi],
                start=True,
                stop=True,
            )
        nc.scalar.activation(
            pt[:, :, (qlo - 128 * j):(qhi - 128 * j)],
            ps[:, :, lo:(qhi - 512 * qh)],
            AF.Exp,
            scale=0.125,
        )
        if qh == j // 4:
            # zero the q < k upper triangle of the diagonal chunk (both
            # heads in one mul via a broadcast AP over the head dim).
            # GpSimd has no PSUM port but pt is SBUF-only, so this runs
            # there, off the DVE queue.
            mb = bass.AP(tensor=mask_t.tensor, offset=mask_t.offset,
                         ap=[mask_t.ap[0], [0, 2], [1, 128]])
            nc.gpsimd.tensor_mul(pt[:, :, 0:128], pt[:, :, 0:128], mb)

    def s_exp(p, j):
        # scores^T for pair p, key chunk j (both heads), all q blocks
        for qh in range(j // 4, 4):
            score_block(p, j, qh)

    pv_ps = {}

    def pv_mm(p, hl, qb4, jp_lo, jp_hi):
        # rank-update chunk range [jp_lo, jp_hi] of the PV accumulation for
        # head (p, hl), q block qb4. jp_hi == 4*qb4+3 closes the group.
        last = 4 * qb4 + 3
        if jp_lo == 0:
            pv_ps[(p, hl, qb4)] = ps_pv.tile(
                [128, QB], F32, tag="pv", name=f"ps_pv{p}{hl}{qb4}")
        ps = pv_ps[(p, hl, qb4)]
        for jp in range(jp_lo, jp_hi + 1):
            pt = pt_tiles[(p, jp)]
            qlo = max(qb4 * QB, 128 * jp)
            qhi = qb4 * QB + QB
            nc.tensor.matmul(
                ps[:, (qlo - qb4 * QB):(qhi - qb4 * QB)],
                lhsT=vnat[:, p, jp, hl, :],
                rhs=pt[:, hl, (qlo - 128 * jp):(qhi - 128 * jp)],
                start=(jp == 0),
                stop=(jp == last),
            )

    def pv_fin(p, hl, qb4):
        # normalize: y = psum_y * (1 / psum_denom), denom rows live on the
        # opposite partition half.  reciprocal_approx_fast is one custom-DVE
        # op whose cost scales with free size only; its 64-channel write may
        # target either partition half, so no reshape/bounce is needed.
        ps = pv_ps.pop((p, hl, qb4))
        ysl = slice(64 * hl, 64 * hl + 64)
        dsl = slice(64 - 64 * hl, 128 - 64 * hl)
        rq = rt_pool.tile([128, QB], F32, tag="rq", name="rq")
        # the custom-DVE recip only routes correctly when anchored at
        # partition 0, so run it over all 128 partitions: the y half of rq
        # is garbage (recip of unnormalized y) but never read. The
        # tensor_mul reads the denominator half cross-base, which is legal
        # when in0 is PSUM (walrus only forbids two cross-base SB inputs).
        nc.vector.reciprocal_approx_fast(out=rq, in_=ps)
        nc.vector.tensor_mul(
            yT[ysl, p, qb4 * QB:(qb4 + 1) * QB], ps[ysl, :], rq[dsl, :]
        )

    def pv_unit(p, hl, qb4):
        pv_mm(p, hl, qb4, 0, 4 * qb4 + 3)
        pv_fin(p, hl, qb4)

    outT_v = outT.rearrange("(n p) t -> p n t", p=128)

    def proj_qb(qb4, split_out=False, nf2s=range(4)):
        # final projection for one q block (needs yT of both pairs for it).
        # evacuation alternates DVE / ACT so neither queue paces the PE.
        # split_out (used for the last block) issues per-nf output DMAs on
        # two queues so the final transfers overlap instead of serializing.
        qsl = slice(qb4 * QB, (qb4 + 1) * QB)
        for nf2 in nf2s:
            ob = osb_pool.tile([128, 2, QB], BF16, tag="osb", name="ob")
            for sub in range(2):
                nf = nf2 * 2 + sub
                ps = ps_q.tile([128, QB], F32, tag="q", name="ps_o")
                for kc in range(2):
                    nc.tensor.matmul(
                        ps,
                        lhsT=Wp_t[:, kc, nf * 128:(nf + 1) * 128],
                        rhs=yT[:, kc, qsl],
                        start=(kc == 0),
                        stop=(kc == 1),
                    )
                # ACT paces pair1 (it owns all the exps), and DVE alone
                # keeps up with proj evacuation (0.69us/copy vs 0.86us of
                # matmul per psum tile), so everything goes to DVE.
                nc.vector.tensor_copy(ob[:, sub, :], ps)
                if split_out:
                    q = nc.sync if sub == 0 else nc.gpsimd
                    q.dma_start(out=outT_v[:, nf:nf + 1, qsl],
                                in_=ob[:, sub:sub + 1, :])
            if not split_out:
                nc.sync.dma_start(out=outT_v[:, nf2 * 2:nf2 * 2 + 2, qsl], in_=ob)

    # ---------------- emission schedule ----------------
    # pair-0 runs q-wave-major: wave w computes the qkv parts for q block
    # w+1 (prefetch, paced by the x DMA waves), the score blocks whose q
    # column is w (for every key chunk j <= 4w+3), the V transposes for the
    # wave's new key chunks, and the wave's PV units. This lets real PE
    # work start as soon as W + x[qb0] land instead of waiting for all of
    # x (scores for q block w only need Q0[:, qb w]).
    with nc.named_scope("head"):
        qkv_part(0, 0)
        qkv_part(1, 0)
        qkv_part(2, 0)
    with nc.named_scope("pair0"):
        for w in range(4):
            # all of this wave's score columns (new diagonal chunks first,
            # then the older chunks' column w)
            for j in range(4 * w, 4 * w + 4):
                score_block(0, j, w)
            for j in range(4 * w):
                score_block(0, j, w)
            # pair-1 qkv parts (need the late-arriving W second half) fill
            # the PE while ACT catches up on the wave's exps
            if w == 1:
                qkv_part(3, 0)          # Q1
            elif w == 2:
                qkv_part(3, 1)
                qkv_part(3, 2)
            elif w == 3:
                qkv_part(3, 3)
                qkv_part(4, 0)          # K1
                qkv_part(5, 0)          # V1
            for jt in range(4 * w, 4 * w + 4):
                vtrans1(0, jt)
            # junk bridges the PE to the wave's last exps (ACT runs just
            # behind); an unbridged sub-2us wait would reset the clock ramp
            junkw = ps_pv.tile([128, 256], F32, tag="pv", name="junkw")
            for _ in range(14 if w == 0 else 5):
                nc.tensor.matmul(junkw, lhsT=wtile[:, 0:128],
                                 rhs=wtile[:, 0:256], start=True, stop=True)
            pv_unit(0, 0, w)
            pv_unit(0, 1, w)
            # prefetch of the next wave's qkv parts comes last: its x wave
            # is the latest-arriving input this wave touches
            if w < 3:
                qkv_part(0, w + 1)
                qkv_part(1, w + 1)
                qkv_part(2, w + 1)
        # bleed pair-1's first two score chunks into pair-0's tail: ACT is
        # ~60% busy here but ~85% busy through pair-1, so this smooths the
        # exp stream across the phase boundary
        s_exp(1, 0)
        s_exp(1, 1)
        s_exp(1, 2)
    # pair-1 scores/exp run k-major (all inputs resident by now),
    # interleaved with the remaining K1/V1 qkv parts, V1 transposes,
    # pair-1 PV units, and the final projection (per q block, as soon as
    # both pairs' yT for that block is done)
    with nc.named_scope("pair1"):
        for j in range(16):
            if j >= 3:
                s_exp(1, j)
            if j < 3:
                qkv_part(4, j + 1)      # K1 tail
                qkv_part(5, j + 1)      # V1 tail
            if 1 <= j <= 8:
                vtrans1(1, 2 * j - 2)
                vtrans1(1, 2 * j - 1)
            if j >= 4 and j % 4 == 0:
                pv_unit(1, 0, j // 4 - 1)
            if j >= 5 and j % 4 == 1:
                pv_unit(1, 1, j // 4 - 1)
            # proj is spread over two j slots: its psum/evac rotation then
            # drains behind the next chunk's score matmuls instead of
            # stalling the PE
            if j >= 6 and j % 4 == 2:
                with nc.named_scope("proj"):
                    proj_qb(j // 4 - 1, nf2s=range(0, 2))
            if j >= 7 and j % 4 == 3:
                with nc.named_scope("proj"):
                    proj_qb(j // 4 - 1, nf2s=range(2, 4))
            if j == 14:
                pv_mm(1, 0, 3, 0, 11)
            if j == 15:
                pv_mm(1, 1, 3, 0, 11)
    with nc.named_scope("tail"):
        # both tails' matmuls back-to-back so the DVE fins overlap the PE;
        # the first two proj psum groups start on their pair-0 half (ready
        # since wave 3) while the fins run, then accumulate pair-1.
        pv_mm(1, 0, 3, 12, 15)
        pv_mm(1, 1, 3, 12, 15)
        qsl = slice(3 * QB, 4 * QB)
        ps01 = []
        for nf in range(2):
            ps = ps_q.tile([128, QB], F32, tag="q", name=f"ps_t{nf}")
            nc.tensor.matmul(ps, lhsT=Wp_t[:, 0, nf * 128:(nf + 1) * 128],
                             rhs=yT[:, 0, qsl], start=True, stop=False)
            ps01.append(ps)
        pv_fin(1, 0, 3)
        pv_fin(1, 1, 3)
        # keep the PE ticking through the ~2us DVE fin chain: an idle gap
        # here would drop the clock to the mid p-state for the entire final
        # projection. (Fresh tile from the now-idle score pool — the "q"
        # slots hold the open ps01 accumulations.)
        junk2 = ps_s.tile([128, 256], F32, tag="s", name="junk2")
        for _ in range(17):
            nc.tensor.matmul(junk2, lhsT=wtile[:, 0:128],
                             rhs=wtile[:, 0:256], start=True, stop=True)
        ob = osb_pool.tile([128, 2, QB], BF16, tag="osb", name="ob_t")
        for nf in range(2):
            nc.tensor.matmul(ps01[nf], lhsT=Wp_t[:, 1, nf * 128:(nf + 1) * 128],
                             rhs=yT[:, 1, qsl], start=False, stop=True)
            if nf == 0:
                nc.vector.tensor_copy(ob[:, nf, :], ps01[nf])
            else:
                nc.scalar.copy(ob[:, nf, :], ps01[nf])
            q = nc.sync if nf == 0 else nc.gpsimd
            q.dma_start(out=outT_v[:, nf:nf + 1, qsl], in_=ob[:, nf:nf + 1, :])
        for nf2 in range(1, 4):
            ob = osb_pool.tile([128, 2, QB], BF16, tag="osb", name="ob")
            for sub in range(2):
                nf = nf2 * 2 + sub
                ps = ps_q.tile([128, QB], F32, tag="q", name="ps_o")
                for kc in range(2):
                    nc.tensor.matmul(
                        ps,
                        lhsT=Wp_t[:, kc, nf * 128:(nf + 1) * 128],
                        rhs=yT[:, kc, qsl],
                        start=(kc == 0),
                        stop=(kc == 1),
                    )
                if sub == 0:
                    nc.vector.tensor_copy(ob[:, sub, :], ps)
                else:
                    nc.scalar.copy(ob[:, sub, :], ps)
                q = nc.sync if sub == 0 else nc.gpsimd
                q.dma_start(out=outT_v[:, nf:nf + 1, qsl],
                            in_=ob[:, sub:sub + 1, :])
    if _DEBUG_DUMPS:
        qkvT_d = nc.dram_tensor("qkvT_d", [128, 6, T], BF16,
                                kind="ExternalOutput").ap()
        yT_d = nc.dram_tensor("yT_d", [128, 2, T], BF16,
                              kind="ExternalOutput").ap()
        vnat_d = nc.dram_tensor("vnat_d", [128, 2, 16, 2, 128], BF16,
                                kind="ExternalOutput").ap()
        nc.sync.dma_start(out=qkvT_d, in_=qkvT)
        nc.sync.dma_start(out=yT_d, in_=yT)
        nc.sync.dma_start(out=vnat_d, in_=vnat)
    ctx.close()


# ---------------------------------------------------------------------------
# host-side wrapper
# ---------------------------------------------------------------------------

_NC_CACHE = {}


def _get_nc():
    if "nc" not in _NC_CACHE:
        _NC_CACHE["nc"] = build_kernel()
    return _NC_CACHE["nc"]


def make_in_maps(x, W_attn, b_attn, W_proj, b_proj):
    # multiplicative causal mask for the diagonal chunk, [k, q]: 1 where q >= k
    mask_np = np.triu(np.ones((128, 128), np.float32)).astype(bf16)
    ident_np = np.eye(128, dtype=np.float32).astype(bf16)
    in_maps = []
    for core in range(N_CORES):
        b = core // 4
        g = core % 4
        # pair-major local column order: [Q_p0 K_p0 V_p0 Q_p1 K_p1 V_p1]
        base = 256 * g
        cols = np.r_[base:base + 128,
                     1024 + base:1024 + base + 128,
                     2048 + base:2048 + base + 128,
                     base + 128:base + 256,
                     1024 + base + 128:1024 + base + 256,
                     2048 + base + 128:2048 + base + 256]
        in_maps.append({
            "xT": np.ascontiguousarray(x[b].T).astype(bf16),
            "W": np.ascontiguousarray(W_attn[:, cols]).astype(bf16),
            "bcols": np.ascontiguousarray(
                b_attn[cols].reshape(6, 128).T).astype(np.float32),
            "Wp": np.ascontiguousarray(
                W_proj[256 * g:256 * g + 256, :]).astype(bf16),
            "mask": mask_np,
            "ident": ident_np,
        })
    return in_maps


def kernel(x, W_attn, b_attn, W_proj, b_proj, _trace=False, _trace_kwargs=None):
    x = np.asarray(x, np.float32)
    W_attn = np.asarray(W_attn, np.float32)
    b_attn = np.asarray(b_attn, np.float32)
    W_proj = np.asarray(W_proj, np.float32)
    b_proj = np.asarray(b_proj, np.float32)

    nc = _get_nc()
    in_maps = make_in_maps(x, W_attn, b_attn, W_proj, b_proj)
    res = run_bass_kernel_spmd(
        nc, in_maps, core_ids=list(range(N_CORES)), trace=_trace,
        **(_trace_kwargs or {}),
    )
    B = x.shape[0]
    out = np.zeros((B, T, C), np.float32)
    for core in range(N_CORES):
        b = core // 4
        out[b] += np.asarray(res.results[core]["outT"], np.float32).T
    out += b_proj[None, None, :]
    if _trace:
        kernel._last_results = res
    return out


if __name__ == "__main__":
    # smoke test: build only
    nc = build_kernel()
    print("built ok")


# revision 47
# speedup vs baseline: 1.0157x; 1.0157x over previous
"""Trainium2 Bass kernel for causal self-attention (dense transformer block attn).

Reference computation (per batch b):
    qkv = x @ W_attn + b_attn ; split into per-head Q, K, V (16 heads, hs=64)
    att = softmax(mask(Q K^T / sqrt(hs))) ; y = att @ V ; out = y @ W_proj + b_proj

Sharding (8 cores): data parallel on B (2) x tensor parallel on head groups
(4 groups of 4 heads, Megatron-style column/row split of W_attn / W_proj).
Each core computes a partial out^T [1024, 2048] (bf16); host sums the 4
partials per batch, adds b_proj and transposes.

Core kernel layout notes:
  - Everything on-chip is transposed: x^T, qkv^T ([feature, T]), scores are
    computed as S^T = K Q^T with k-positions on partitions so that the PV
    matmul needs no transposes (P^T is the moving operand, V natural the
    stationary).
  - Softmax denominator: the PV stationary is [V | ones] (or [ones | V]) so
    the other 64 partitions of the PV psum accumulate 64 copies of
    sum_k P[q,k]. A single custom-DVE reciprocal_approx_fast reads those
    partitions and writes the reciprocal onto the y-row partition half
    (64-channel DVE ops may write either partition half regardless of the
    source half), then one tensor_mul normalizes during psum evacuation.
  - exp() runs on ScalarE straight out of PSUM in wide [128, 2, <=512]
    instructions (two heads at once) to amortize the ACT overhead.
  - The PE p-state ramp (0.65 -> 1.2 -> 2.4 GHz after 3us of *continuous*
    work) makes every PE idle gap cost ~2x its length: the schedule keeps
    the PE queue dense front-to-back and warms the array with junk matmuls
    while the first input DMAs land.
"""

import numpy as np
import ml_dtypes

import concourse.bass as bass
import concourse.tile as tile
import concourse.mybir as mybir
from concourse import bacc
from concourse.bass_utils import run_bass_kernel_spmd

BF16 = mybir.dt.bfloat16
F32 = mybir.dt.float32
AF = mybir.ActivationFunctionType

T = 2048          # sequence length
C = 1024          # model dim
HPC = 4           # heads per core
HS = 64           # head size
NF = 3 * HPC * HS  # per-core qkv features (768)
N_CORES = 8
QB = 512          # q block (psum bank of f32)

bf16 = ml_dtypes.bfloat16

_DEBUG_DUMPS = False


def build_kernel():
    nc = bacc.Bacc("TRN2", target_bir_lowering=False, debug=False)

    xT = nc.dram_tensor("xT", [C, T], BF16, kind="ExternalInput").ap()
    W = nc.dram_tensor("W", [C, NF], BF16, kind="ExternalInput").ap()
    bcols = nc.dram_tensor("bcols", [128, 6], F32, kind="ExternalInput").ap()
    Wp = nc.dram_tensor("Wp", [HPC * HS, C], BF16, kind="ExternalInput").ap()
    mask = nc.dram_tensor("mask", [128, 128], BF16, kind="ExternalInput").ap()
    ident = nc.dram_tensor("ident", [128, 128], BF16, kind="ExternalInput").ap()
    outT = nc.dram_tensor("outT", [C, T], BF16, kind="ExternalOutput").ap()

    with tile.TileContext(nc) as tc:
        _emit(nc, tc, xT, W, bcols, Wp, mask, ident, outT)
    nc.compile()
    return nc


def _emit(nc, tc, xT, W, bcols, Wp, mask, ident, outT):
    from contextlib import ExitStack

    ctx = ExitStack()
    consts = ctx.enter_context(tc.tile_pool(name="consts", bufs=1))
    pt_pool = ctx.enter_context(tc.tile_pool(name="pt", bufs=1))
    rt_pool = ctx.enter_context(tc.tile_pool(name="rt", bufs=2))
    # 4 bufs: an ob tile is held until its output DMA transfer completes
    # (~1-2.5us), so depth 2 would pace the projection evacuations
    osb_pool = ctx.enter_context(tc.tile_pool(name="osb", bufs=4))
    ps_s = ctx.enter_context(tc.tile_pool(name="ps_s", bufs=2, space="PSUM"))
    ps_pv = ctx.enter_context(tc.tile_pool(name="ps_pv", bufs=2, space="PSUM"))
    ps_q = ctx.enter_context(tc.tile_pool(name="ps_q", bufs=2, space="PSUM"))

    # ---------------- constant / input loads ----------------
    xT_v = xT.rearrange("(c p) t -> p c t", p=128)
    xT_t = consts.tile([128, 8, T], BF16, tag="xT", name="xT_t")
    W_v = W.rearrange("(c p) n -> p c n", p=128)
    W_t = consts.tile([128, 8, NF], BF16, tag="W", name="W_t")
    mask_t = consts.tile([128, 128], BF16, tag="mask", name="mask_t")
    ident_t = consts.tile([128, 128], BF16, tag="ident", name="id_t")
    b_t = consts.tile([128, 6], F32, tag="b", name="b_t")
    Wp_t = consts.tile([128, 2, C], BF16, tag="Wp", name="Wp_t")

    qkvT = consts.tile([128, 6, T], BF16, tag="qkvT", name="qkvT")
    # vnat[p, pair, j, hl, col]: PV stationary tiles. hl=0: [V | ones],
    # hl=1: [ones | V] so that y lands on the partitions matching yT layout.
    vnat = consts.tile([128, 2, 16, 2, 128], BF16, tag="vnat", name="vnat")
    yT = consts.tile([128, 2, T], BF16, tag="yT", name="yT")

    # warm up the ACT exp table early so the ~1.3us load overlaps the lead-in
    warm = consts.tile([128, 8], F32, tag="warm", name="warm")
    nc.vector.memset(warm, 0.0)
    nc.scalar.activation(warm, warm, AF.Exp, scale=1.0)
    wtile = consts.tile([128, QB], BF16, tag="wtile", name="wtile")
    nc.vector.memset(wtile, 0.0)
    nc.vector.memset(vnat[:, :, :, 0, 64:128], 1.0)
    nc.vector.memset(vnat[:, :, :, 1, 0:64], 1.0)

    # The Scalar queue's DMA path is much slower than Sync/GpSimd and its
    # ring blocks later issues, so it only carries the tiny constants.
    nc.scalar.dma_start(out=mask_t, in_=mask)
    nc.scalar.dma_start(out=ident_t, in_=ident)
    nc.scalar.dma_start(out=b_t, in_=bcols)
    # Input streaming, ordered by first use. W is pair-major (host-side
    # column regroup): its first half (Q0/K0/V0) plus x[qb0] gate wave 0;
    # everything else overlaps compute. Sync takes c chunks 0-3, GpSimd
    # 4-7, both walking [W-half0, x qb0, x qb1, W-half1, x qb2, x qb3].
    CG = [(0, 4), (4, 8)]
    QUEUES = [nc.sync, nc.gpsimd]

    def xdma(c0, c1, qb4, q):
        q.dma_start(out=xT_t[:, c0:c1, qb4 * QB:(qb4 + 1) * QB],
                    in_=xT_v[:, c0:c1, qb4 * QB:(qb4 + 1) * QB])

    for (c0, c1), q in zip(CG, QUEUES):
        q.dma_start(out=W_t[:, c0:c1, 0:NF // 2], in_=W_v[:, c0:c1, 0:NF // 2])
        xdma(c0, c1, 0, q)
        xdma(c0, c1, 1, q)
        q.dma_start(out=W_t[:, c0:c1, NF // 2:NF], in_=W_v[:, c0:c1, NF // 2:NF])
        xdma(c0, c1, 2, q)
        xdma(c0, c1, 3, q)
    # Wp is only needed at the first proj (~halfway in); the slow Scalar
    # DMA path delivers it comfortably by then without loading Sync/GpSimd
    nc.scalar.dma_start(out=Wp_t, in_=Wp.rearrange("(k p) n -> p k n", p=128))

    # ~7us of narrow junk matmuls: the PE p-state ramps to full clock while
    # the first input wave (W + x qb0) lands, and the fine granularity
    # (107ns each at full clock) means the first real matmul preempts with
    # negligible delay.
    junk = ps_q.tile([128, QB], F32, tag="q", name="junk")
    for _ in range(57):
        nc.tensor.matmul(junk[:, 0:256], lhsT=wtile[:, 0:128],
                         rhs=wtile[:, 0:256], start=True, stop=True)

    # ---------------- phase helpers ----------------
    def qkv_part(nf, qb4):
        # one q block of qkv^T[nf*128:(nf+1)*128, :]  (+ bias on evac)
        ps = ps_q.tile([128, QB], F32, tag="q", name="ps_qkv")
        for c in range(8):
            nc.tensor.matmul(
                ps,
                lhsT=W_t[:, c, nf * 128:(nf + 1) * 128],
                rhs=xT_t[:, c, qb4 * QB:(qb4 + 1) * QB],
                start=(c == 0),
                stop=(c == 7),
            )
        nc.vector.tensor_scalar_add(
            qkvT[:, nf, qb4 * QB:(qb4 + 1) * QB], ps, b_t[:, nf:nf + 1]
        )

    def vtrans1(p, jt):
        # one V^T chunk (qkvT[:, 3p+2, jt]) -> natural V in vnat[:, p, jt]
        pst = ps_q.tile([128, 128], BF16, tag="q", name="ps_vt")
        nc.tensor.transpose(pst, qkvT[:, 3 * p + 2, jt * 128:(jt + 1) * 128], ident_t)
        # single strided copy: psum cols [0:64|64:128] -> vnat
        # [jt, 0, 0:64] and [jt, 1, 64:128]
        v0 = vnat[:, p, jt, 0, 0:64]
        dst = bass.AP(tensor=v0.tensor, offset=v0.offset,
                      ap=[v0.ap[0], [192, 2], [1, 64]])
        s0 = pst[:, 0:64]
        src = bass.AP(tensor=s0.tensor, offset=s0.offset,
                      ap=[s0.ap[0], [64, 2], [1, 64]])
        nc.vector.tensor_copy(dst, src)

    pt_tiles = {}

    def score_block(p, j, qh):
        # one [128 k, <=512 q] score block of S^T chunk j (both heads),
        # exp'd into the (p, j) PT tile. Blocks of a chunk may be emitted
        # across different q waves (pair0) or together (pair1).
        wj = T - 128 * j
        if (p, j) not in pt_tiles:
            pt_tiles[(p, j)] = pt_pool.tile([128, 2, wj], BF16, tag=f"pt{j}",
                                            name=f"pt_{p}_{j}", bufs=1)
        pt = pt_tiles[(p, j)]
        qlo = max(128 * j, 512 * qh)
        qhi = 512 * (qh + 1)
        if qlo >= qhi:
            return
        lo = qlo - 512 * qh
        ps = ps_s.tile([128, 2, 512], F32, tag="s", name="ps_s_t")
        for hl in range(2):
            nc.tensor.matmul(
                ps[:, hl, lo:(qhi - 512 * qh)],
                lhsT=qkvT[64 * hl:64 * hl + 64, 3 * p + 1, j * 128:(j + 1) * 128],
                rhs=qkvT[64 * hl:64 * hl + 64, 3 * p, qlo:qhi],
                start=True,
                stop=True,
            )
        nc.scalar.activation(
            pt[:, :, (qlo - 128 * j):(qhi - 128 * j)],
            ps[:, :, lo:(qhi - 512 * qh)],
            AF.Exp,
            scale=0.125,
        )
        if qh == j // 4:
            # zero the q < k upper triangle of the diagonal chunk (both
            # heads in one mul via a broadcast AP over the head dim).
            # GpSimd has no PSUM port but pt is SBUF-only, so this runs
            # there, off the DVE queue.
            mb = bass.AP(tensor=mask_t.tensor, offset=mask_t.offset,
                         ap=[mask_t.ap[0], [0, 2], [1, 128]])
            nc.gpsimd.tensor_mul(pt[:, :, 0:128], pt[:, :, 0:128], mb)

    def s_exp(p, j):
        # scores^T for pair p, key chunk j (both heads), all q blocks
        for qh in range(j // 4, 4):
            score_block(p, j, qh)

    pv_ps = {}

    def pv_mm(p, hl, qb4, jp_lo, jp_hi):
        # rank-update chunk range [jp_lo, jp_hi] of the PV accumulation for
        # head (p, hl), q block qb4. jp_hi == 4*qb4+3 closes the group.
        last = 4 * qb4 + 3
        if jp_lo == 0:
            pv_ps[(p, hl, qb4)] = ps_pv.tile(
                [128, QB], F32, tag="pv", name=f"ps_pv{p}{hl}{qb4}")
        ps = pv_ps[(p, hl, qb4)]
        for jp in range(jp_lo, jp_hi + 1):
            pt = pt_tiles[(p, jp)]
            qlo = max(qb4 * QB, 128 * jp)
            qhi = qb4 * QB + QB
            nc.tensor.matmul(
                ps[:, (qlo - qb4 * QB):(qhi - qb4 * QB)],
                lhsT=vnat[:, p, jp, hl, :],
                rhs=pt[:, hl, (qlo - 128 * jp):(qhi - 128 * jp)],
                start=(jp == 0),
                stop=(jp == last),
            )

    def pv_fin(p, hl, qb4):
        # normalize: y = psum_y * (1 / psum_denom), denom rows live on the
        # opposite partition half.  reciprocal_approx_fast is one custom-DVE
        # op whose cost scales with free size only; its 64-channel write may
        # target either partition half, so no reshape/bounce is needed.
        ps = pv_ps.pop((p, hl, qb4))
        ysl = slice(64 * hl, 64 * hl + 64)
        dsl = slice(64 - 64 * hl, 128 - 64 * hl)
        rq = rt_pool.tile([128, QB], F32, tag="rq", name="rq")
        # the custom-DVE recip only routes correctly when anchored at
        # partition 0, so run it over all 128 partitions: the y half of rq
        # is garbage (recip of unnormalized y) but never read. The
        # tensor_mul reads the denominator half cross-base, which is legal
        # when in0 is PSUM (walrus only forbids two cross-base SB inputs).
        nc.vector.reciprocal_approx_fast(out=rq, in_=ps)
        nc.vector.tensor_mul(
            yT[ysl, p, qb4 * QB:(qb4 + 1) * QB], ps[ysl, :], rq[dsl, :]
        )

    def pv_unit(p, hl, qb4):
        pv_mm(p, hl, qb4, 0, 4 * qb4 + 3)
        pv_fin(p, hl, qb4)

    outT_v = outT.rearrange("(n p) t -> p n t", p=128)

    def proj_qb(qb4, split_out=False, nf2s=range(4)):
        # final projection for one q block (needs yT of both pairs for it).
        # evacuation alternates DVE / ACT so neither queue paces the PE.
        # split_out (used for the last block) issues per-nf output DMAs on
        # two queues so the final transfers overlap instead of serializing.
        qsl = slice(qb4 * QB, (qb4 + 1) * QB)
        for nf2 in nf2s:
            ob = osb_pool.tile([128, 2, QB], BF16, tag="osb", name="ob")
            for sub in range(2):
                nf = nf2 * 2 + sub
                ps = ps_q.tile([128, QB], F32, tag="q", name="ps_o")
                for kc in range(2):
                    nc.tensor.matmul(
                        ps,
                        lhsT=Wp_t[:, kc, nf * 128:(nf + 1) * 128],
                        rhs=yT[:, kc, qsl],
                        start=(kc == 0),
                        stop=(kc == 1),
                    )
                # ACT paces pair1 (it owns all the exps), and DVE alone
                # keeps up with proj evacuation (0.69us/copy vs 0.86us of
                # matmul per psum tile), so everything goes to DVE.
                nc.vector.tensor_copy(ob[:, sub, :], ps)
                if split_out:
                    q = nc.sync if sub == 0 else nc.gpsimd
                    q.dma_start(out=outT_v[:, nf:nf + 1, qsl],
                                in_=ob[:, sub:sub + 1, :])
            if not split_out:
                nc.sync.dma_start(out=outT_v[:, nf2 * 2:nf2 * 2 + 2, qsl], in_=ob)

    # ---------------- emission schedule ----------------
    # pair-0 runs q-wave-major: wave w computes the qkv parts for q block
    # w+1 (prefetch, paced by the x DMA waves), the score blocks whose q
    # column is w (for every key chunk j <= 4w+3), the V transposes for the
    # wave's new key chunks, and the wave's PV units. This lets real PE
    # work start as soon as W + x[qb0] land instead of waiting for all of
    # x (scores for q block w only need Q0[:, qb w]).
    with nc.named_scope("head"):
        qkv_part(0, 0)
        qkv_part(1, 0)
        qkv_part(2, 0)
    with nc.named_scope("pair0"):
        for w in range(4):
            # all of this wave's score columns (new diagonal chunks first,
            # then the older chunks' column w)
            for j in range(4 * w, 4 * w + 4):
                score_block(0, j, w)
            for j in range(4 * w):
                score_block(0, j, w)
            # pair-1 qkv parts (need the late-arriving W second half) fill
            # the PE while ACT catches up on the wave's exps
            if w == 1:
                qkv_part(3, 0)          # Q1
            elif w == 2:
                qkv_part(3, 1)
                qkv_part(3, 2)
            elif w == 3:
                qkv_part(3, 3)
                qkv_part(4, 0)          # K1
                qkv_part(5, 0)          # V1
            for jt in range(4 * w, 4 * w + 4):
                vtrans1(0, jt)
            # junk bridges the PE to the wave's last exps (ACT runs just
            # behind); an unbridged sub-2us wait would reset the clock ramp
            junkw = ps_pv.tile([128, 256], F32, tag="pv", name="junkw")
            for _ in range(14 if w == 0 else 5):
                nc.tensor.matmul(junkw, lhsT=wtile[:, 0:128],
                                 rhs=wtile[:, 0:256], start=True, stop=True)
            pv_unit(0, 0, w)
            pv_unit(0, 1, w)
            # prefetch of the next wave's qkv parts comes last: its x wave
            # is the latest-arriving input this wave touches
            if w < 3:
                qkv_part(0, w + 1)
                qkv_part(1, w + 1)
                qkv_part(2, w + 1)
        # bleed pair-1's first two score chunks into pair-0's tail: ACT is
        # ~60% busy here but ~85% busy through pair-1, so this smooths the
        # exp stream across the phase boundary
        s_exp(1, 0)
        s_exp(1, 1)
        s_exp(1, 2)
    # pair-1 scores/exp run k-major (all inputs resident by now),
    # interleaved with the remaining K1/V1 qkv parts, V1 transposes,
    # pair-1 PV units, and the final projection (per q block, as soon as
    # both pairs' yT for that block is done)
    with nc.named_scope("pair1"):
        for j in range(16):
            if j >= 3:
                s_exp(1, j)
            if j < 3:
                qkv_part(4, j + 1)      # K1 tail
                qkv_part(5, j + 1)      # V1 tail
            if 1 <= j <= 8:
                vtrans1(1, 2 * j - 2)
                vtrans1(1, 2 * j - 1)
            if j >= 4 and j % 4 == 0:
                pv_unit(1, 0, j // 4 - 1)
            if j >= 5 and j % 4 == 1:
                pv_unit(1, 1, j // 4 - 1)
            # proj is spread over two j slots: its psum/evac rotation then
            # drains behind the next chunk's score matmuls instead of
            # stalling the PE
            if j >= 6 and j % 4 == 2:
                with nc.named_scope("proj"):
                    proj_qb(j // 4 - 1, nf2s=range(0, 2))
            if j >= 7 and j % 4 == 3:
                with nc.named_scope("proj"):
                    proj_qb(j // 4 - 1, nf2s=range(2, 4))
            if j == 14:
                pv_mm(1, 0, 3, 0, 11)
            if j == 15:
                pv_mm(1, 1, 3, 0, 11)
    with nc.named_scope("tail"):
        # both tails' matmuls back-to-back so the DVE fins overlap the PE;
        # the first two proj psum groups start on their pair-0 half (ready
        # since wave 3) while the fins run, then accumulate pair-1.
        pv_mm(1, 0, 3, 12, 15)
        pv_mm(1, 1, 3, 12, 15)
        qsl = slice(3 * QB, 4 * QB)
        ps01 = []
        for nf in range(2):
            ps = ps_q.tile([128, QB], F32, tag="q", name=f"ps_t{nf}")
            nc.tensor.matmul(ps, lhsT=Wp_t[:, 0, nf * 128:(nf + 1) * 128],
                             rhs=yT[:, 0, qsl], start=True, stop=False)
            ps01.append(ps)
        pv_fin(1, 0, 3)
        pv_fin(1, 1, 3)
        # keep the PE ticking through the ~2us DVE fin chain: an idle gap
        # here would drop the clock to the mid p-state for the entire final
        # projection. (Fresh tile from the now-idle score pool — the "q"
        # slots hold the open ps01 accumulations.)
        junk2 = ps_s.tile([128, 256], F32, tag="s", name="junk2")
        for _ in range(17):
            nc.tensor.matmul(junk2, lhsT=wtile[:, 0:128],
                             rhs=wtile[:, 0:256], start=True, stop=True)
        ob = osb_pool.tile([128, 2, QB], BF16, tag="osb", name="ob_t")
        for nf in range(2):
            nc.tensor.matmul(ps01[nf], lhsT=Wp_t[:, 1, nf * 128:(nf + 1) * 128],
                             rhs=yT[:, 1, qsl], start=False, stop=True)
            if nf == 0:
                nc.vector.tensor_copy(ob[:, nf, :], ps01[nf])
            else:
                nc.scalar.copy(ob[:, nf, :], ps01[nf])
            q = nc.sync if nf == 0 else nc.gpsimd
            q.dma_start(out=outT_v[:, nf:nf + 1, qsl], in_=ob[:, nf:nf + 1, :])
        for nf2 in range(1, 4):
            ob = osb_pool.tile([128, 2, QB], BF16, tag="osb", name="ob")
            for sub in range(2):
                nf = nf2 * 2 + sub
                ps = ps_q.tile([128, QB], F32, tag="q", name="ps_o")
                for kc in range(2):
                    nc.tensor.matmul(
                        ps,
                        lhsT=Wp_t[:, kc, nf * 128:(nf + 1) * 128],
                        rhs=yT[:, kc, qsl],
                        start=(kc == 0),
                        stop=(kc == 1),
                    )
                if sub == 0:
                    nc.vector.tensor_copy(ob[:, sub, :], ps)
                else:
                    nc.scalar.copy(ob[:, sub, :], ps)
                q = nc.sync if sub == 0 else nc.gpsimd
                q.dma_start(out=outT_v[:, nf:nf + 1, qsl],
                            in_=ob[:, sub:sub + 1, :])
    if _DEBUG_DUMPS:
        qkvT_d = nc.dram_tensor("qkvT_d", [128, 6, T], BF16,
                                kind="ExternalOutput").ap()
        yT_d = nc.dram_tensor("yT_d", [128, 2, T], BF16,
                              kind="ExternalOutput").ap()
        vnat_d = nc.dram_tensor("vnat_d", [128, 2, 16, 2, 128], BF16,
                                kind="ExternalOutput").ap()
        nc.sync.dma_start(out=qkvT_d, in_=qkvT)
        nc.sync.dma_start(out=yT_d, in_=yT)
        nc.sync.dma_start(out=vnat_d, in_=vnat)
    ctx.close()


# ---------------------------------------------------------------------------
# host-side wrapper
# ---------------------------------------------------------------------------

_NC_CACHE = {}


def _get_nc():
    if "nc" not in _NC_CACHE:
        _NC_CACHE["nc"] = build_kernel()
    return _NC_CACHE["nc"]


def make_in_maps(x, W_attn, b_attn, W_proj, b_proj):
    # multiplicative causal mask for the diagonal chunk, [k, q]: 1 where q >= k
    mask_np = np.triu(np.ones((128, 128), np.float32)).astype(bf16)
    ident_np = np.eye(128, dtype=np.float32).astype(bf16)
    in_maps = []
    for core in range(N_CORES):
        b = core // 4
        g = core % 4
        # pair-major local column order: [Q_p0 K_p0 V_p0 Q_p1 K_p1 V_p1]
        base = 256 * g
        cols = np.r_[base:base + 128,
                     1024 + base:1024 + base + 128,
                     2048 + base:2048 + base + 128,
                     base + 128:base + 256,
                     1024 + base + 128:1024 + base + 256,
                     2048 + base + 128:2048 + base + 256]
        in_maps.append({
            "xT": np.ascontiguousarray(x[b].T).astype(bf16),
            "W": np.ascontiguousarray(W_attn[:, cols]).astype(bf16),
            "bcols": np.ascontiguousarray(
                b_attn[cols].reshape(6, 128).T).astype(np.float32),
            "Wp": np.ascontiguousarray(
                W_proj[256 * g:256 * g + 256, :]).astype(bf16),
            "mask": mask_np,
            "ident": ident_np,
        })
    return in_maps


def kernel(x, W_attn, b_attn, W_proj, b_proj, _trace=False, _trace_kwargs=None):
    x = np.asarray(x, np.float32)
    W_attn = np.asarray(W_attn, np.float32)
    b_attn = np.asarray(b_attn, np.float32)
    W_proj = np.asarray(W_proj, np.float32)
    b_proj = np.asarray(b_proj, np.float32)

    nc = _get_nc()
    in_maps = make_in_maps(x, W_attn, b_attn, W_proj, b_proj)
    res = run_bass_kernel_spmd(
        nc, in_maps, core_ids=list(range(N_CORES)), trace=_trace,
        **(_trace_kwargs or {}),
    )
    B = x.shape[0]
    out = np.zeros((B, T, C), np.float32)
    for core in range(N_CORES):
        b = core // 4
        out[b] += np.asarray(res.results[core]["outT"], np.float32).T
    out += b_proj[None, None, :]
    if _trace:
        kernel._last_results = res
    return out


if __name__ == "__main__":
    # smoke test: build only
    nc = build_kernel()
    print("built ok")
